# revision 19
# baseline (speedup 1.0000x reference)
"""Trainium2 Bass kernel for sliding-window GQA attention block (v2).

Reference computation (B=2, S=4096, DIM=1024, H=16 q-heads, KV=2 kv-heads,
D=64, W=256 window):
    q = x@Wq + bq ; k = x@Wk + bk ; v = x@Wv + bv        (GQA repeat kv x8)
    local attention: query t attends keys [t-128, t+128) (zero-padded edges,
    no 1/sqrt(d) scaling), softmax, out = probs@v
    y = out@Wo + bo

Sharding: 8 cores = batch(2) x seq-quarter(4). Each core computes 1024
query rows end-to-end (all 16 heads) from a 1280-row haloed x slice.
No cross-core communication; host pads/transposes/gathers; bo is added
on the host (purely additive after the last matmul).

v2 structure (vs baseline):
  - K/V/Q projections run k-chunk-outer so the PE starts as soon as the
    first xT/wq DMA chunk lands; K is computed directly transposed with
    wk stationary; biases fold into DVE scalar_tensor_tensor ops (no K=1
    bias matmuls).
  - scores computed transposed as before (keys on partitions, kv-halves
    row-packed via tile_position so the two K=64 matmuls overlap).
  - probs@V is FLIPPED: the stationary is [V(64) | ones(64)] so the
    output lands directly in attnT orientation (head-dim on partitions)
    with the softmax denominator replicated across partitions 64-127 --
    no PE transposes, no partition broadcasts. Normalization is a DVE
    reciprocal_approx_fast on the replicated denominator + fused
    multiply into attnT.
  - the middle score chunk (always fully in-window) skips the band-mask
    multiply entirely.
  - out projection accumulates in PSUM and DMAs PSUM->DRAM directly.
"""

import functools
import numpy as np

B, S, DIM = 2, 4096, 1024
H, KV, D = 16, 2, 64
W, HW = 256, 128
NCORES = 8
QT = 4           # sequence quarters
T = S // QT      # 1024 query rows per core
TH = T + 2 * HW  # 1280 haloed rows
NU = TH // 128   # 10 key/value u-tiles


@functools.lru_cache(maxsize=1)
def _build_nc():
    import concourse.bacc as bacc
    import concourse.tile as tile
    from concourse import mybir

    f32 = mybir.dt.float32
    bf16 = mybir.dt.bfloat16
    Exp = mybir.ActivationFunctionType.Exp
    Identity = mybir.ActivationFunctionType.Identity
    MUL = mybir.AluOpType.mult

    nc = bacc.Bacc("TRN2", target_bir_lowering=False, debug=False)

    xT = nc.dram_tensor("xT", [DIM, TH], bf16, kind="ExternalInput")
    wq = nc.dram_tensor("Wq", [DIM, DIM], bf16, kind="ExternalInput")
    wk = nc.dram_tensor("Wk", [DIM, KV * D], bf16, kind="ExternalInput")
    wv = nc.dram_tensor("Wv", [DIM, KV * D], bf16, kind="ExternalInput")
    wo = nc.dram_tensor("Wo", [DIM, DIM], bf16, kind="ExternalInput")
    bqc = nc.dram_tensor("bqc", [128, 8], f32, kind="ExternalInput")
    bk_col = nc.dram_tensor("bk_col", [128, 1], f32, kind="ExternalInput")
    bv_row = nc.dram_tensor("bv_row", [1, KV * D], f32, kind="ExternalInput")
    ind = nc.dram_tensor("ind", [1, TH], bf16, kind="ExternalInput")
    ind_col = nc.dram_tensor("ind_col", [128, NU], f32, kind="ExternalInput")
    out = nc.dram_tensor("out", [T, DIM], bf16, kind="ExternalOutput")

    with tile.TileContext(nc) as tc:
        with tc.tile_pool(name="const", bufs=1) as const, \
             tc.tile_pool(name="w", bufs=1) as wpool, \
             tc.tile_pool(name="act", bufs=1) as actp, \
             tc.tile_pool(name="attn", bufs=2) as attnp, \
             tc.tile_pool(name="ps", bufs=2, space="PSUM") as ps:

            # ---- weight/activation loads ----------------------------------
            # descriptor generation (~650ns per dma_start) serializes per
            # sequencer queue, so the critical first chunks (xT[k], wq[k],
            # then wk) round-robin across three DMA-capable queues; wv/wo
            # and small consts follow.
            qs = [nc.sync, nc.scalar, nc.gpsimd]
            xT_sb, wq_sb, wk_sb, wv_sb, wo_sb = [], [], [], [], []
            for k in range(8):
                t_x = wpool.tile([128, TH], bf16, tag=f"xT{k}", name=f"xT{k}")
                qs[(2 * k) % 3].dma_start(out=t_x,
                                          in_=xT[k * 128:(k + 1) * 128, :])
                xT_sb.append(t_x)
                # only the m0-3 half of wq[k] gates Q-group 0: keep the
                # priority stream lean so the PE never starves mid k-loop
                t_q = wpool.tile([128, DIM], bf16, tag=f"wq{k}", name=f"wq{k}")
                qs[(2 * k + 1) % 3].dma_start(
                    out=t_q[:, 0:512], in_=wq[k * 128:(k + 1) * 128, 0:512])
                wq_sb.append(t_q)
            for k in range(8):
                t_k = wpool.tile([128, KV * D], bf16, tag=f"wk{k}", name=f"wk{k}")
                qs[k % 3].dma_start(out=t_k, in_=wk[k * 128:(k + 1) * 128, :])
                wk_sb.append(t_k)
            for k in range(8):
                qs[(k + 1) % 3].dma_start(
                    out=wq_sb[k][:, 512:1024],
                    in_=wq[k * 128:(k + 1) * 128, 512:1024])
            bq_sb = const.tile([128, 8], f32, tag="bq")
            nc.scalar.dma_start(out=bq_sb, in_=bqc[:, :])
            bkc = const.tile([128, 1], f32, tag="bkc")
            nc.sync.dma_start(out=bkc, in_=bk_col[:, :])
            ind_sb = const.tile([1, TH], bf16, tag="ind")
            nc.gpsimd.dma_start(out=ind_sb, in_=ind[:, :])
            for k in range(8):
                t_v = wpool.tile([128, KV * D], bf16, tag=f"wv{k}", name=f"wv{k}")
                qs[k % 3].dma_start(out=t_v, in_=wv[k * 128:(k + 1) * 128, :])
                wv_sb.append(t_v)
                t_o = wpool.tile([128, DIM], bf16, tag=f"wo{k}", name=f"wo{k}")
                qs[(k + 1) % 3].dma_start(out=t_o,
                                          in_=wo[k * 128:(k + 1) * 128, :])
                wo_sb.append(t_o)
            bvr = const.tile([1, KV * D], f32, tag="bvr")
            nc.scalar.dma_start(out=bvr, in_=bv_row[:, :])
            indc = const.tile([128, NU], f32, tag="indc")
            nc.sync.dma_start(out=indc, in_=ind_col[:, :])
            # broadcast rows for column-varying bias/mask folds
            ind_bc = const.tile([128, TH], bf16, tag="ind_bc")
            nc.gpsimd.partition_broadcast(out_ap=ind_bc, in_ap=ind_sb)
            bv_bc = const.tile([128, KV * D], f32, tag="bv_bc")
            nc.gpsimd.partition_broadcast(out_ap=bv_bc, in_ap=bvr)

            # 0/1 band masks, transposed orientation (key partition r, query
            # col c), full 1024 wide = 8 blocks of 128 (4 head-blocks per
            # kv-half). Chunk j=0 valid where r >= c; j=2 valid where r < c;
            # j=1 is always fully valid and is never masked. Built on the
            # (head-phase-idle) DVE so the gpsimd queue stays free for DMA.
            mA8 = const.tile([128, 512], bf16, tag="mA8")
            mB8 = const.tile([128, 512], bf16, tag="mB8")
            nc.gpsimd.memset(mA8, 1.0)
            nc.gpsimd.memset(mB8, 1.0)
            for blk in range(4):
                nc.gpsimd.affine_select(
                    out=mA8[:, blk * 128:(blk + 1) * 128],
                    in_=mA8[:, blk * 128:(blk + 1) * 128],
                    compare_op=mybir.AluOpType.is_ge,
                    fill=0.0, base=0, pattern=[[-1, 128]],
                    channel_multiplier=1)
                nc.gpsimd.affine_select(
                    out=mB8[:, blk * 128:(blk + 1) * 128],
                    in_=mB8[:, blk * 128:(blk + 1) * 128],
                    compare_op=mybir.AluOpType.is_ge,
                    fill=0.0, base=-1, pattern=[[1, 128]],
                    channel_multiplier=-1)

            # ---- Q projection: qT tile g holds heads (m, m+8) on partition
            # halves for m = 4g..4g+3 (column-permuted Wq does the packing).
            # k-chunk-outer in groups of 4 m so the PE consumes xT/wq DMA
            # chunks as they arrive.
            qT_sb = [actp.tile([128, 4 * T], bf16, tag=f"qT{g}", name=f"qT{g}")
                     for g in range(2)]

            def q_group(grp):                    # m in [4*grp, 4*grp+4)
                pa = [ps.tile([128, 512], f32, tag="A", bufs=8,
                              name=f"qA{grp}{i}") for i in range(8)]
                for k in range(8):
                    for mi in range(4):
                        m = 4 * grp + mi
                        for n in range(2):
                            nc.tensor.matmul(
                                out=pa[2 * mi + n],
                                lhsT=wq_sb[k][:, m * 128:(m + 1) * 128],
                                rhs=xT_sb[k][:, HW + n * 512: HW + (n + 1) * 512],
                                start=(k == 0), stop=(k == 7))
                for mi in range(4):
                    m = 4 * grp + mi
                    off = (m % 4) * T
                    for n in range(2):
                        nc.scalar.activation(
                            out=qT_sb[grp][:, off + n * 512:off + (n + 1) * 512],
                            in_=pa[2 * mi + n],
                            func=Identity, bias=bq_sb[:, m:m + 1], scale=1.0)

            # ---- K projection, directly transposed (kv*64+d on partitions,
            # token on free). wk stationary, xT moving; bias-add and halo
            # zeroing fused into the DVE copy. Emitted between the two Q
            # groups so the PE has work while Q-grp0's ACT copies drain.
            kT_sb = actp.tile([128, TH], bf16, tag="kT")

            def k_proj():
                k_ps = [ps.tile([128, 512], f32, tag="A", bufs=8,
                                name=f"kp{c}") for c in range(3)]
                k_dst = [k_ps[0][:, :], k_ps[1][:, :], k_ps[2][:, 0:256]]
                k_w = [512, 512, 256]
                for k in range(8):
                    for c in range(3):
                        nc.tensor.matmul(
                            out=k_dst[c], lhsT=wk_sb[k],
                            rhs=xT_sb[k][:, c * 512:c * 512 + k_w[c]],
                            start=(k == 0), stop=(k == 7))
                for c in range(3):
                    nc.vector.scalar_tensor_tensor(
                        out=kT_sb[:, c * 512:c * 512 + k_w[c]],
                        in0=k_dst[c], scalar=bkc[:, 0:1],
                        in1=ind_bc[:, c * 512:c * 512 + k_w[c]],
                        op0=mybir.AluOpType.add, op1=MUL)

            q_group(0)
            k_proj()
            q_group(1)

            # ---- V projection (keys on partitions). v_sb u-tile layout per
            # kv-half g: [ones (64) | V (64)]; the 64 ones columns make the
            # flipped probs@[1|V] matmul emit the softmax denominator
            # REPLICATED on output partitions 0-63 (base 0, required by
            # reciprocal_approx_fast). ut-outer / k-inner with one PSUM tile
            # per ut: interleaved accumulation groups must not share a PSUM
            # bank (start=True clears the whole bank).
            v_sb = actp.tile([128, NU * 256], bf16, tag="V")
            v_view = v_sb.rearrange("p (u g c) -> p u g c", u=NU, g=2)
            nc.vector.memset(v_view[:, :, :, 0:64], 1.0)
            bvm = attnp.tile([128, KV * D], f32, tag="bvm", bufs=2)
            for ut in range(NU):
                v_ps = ps.tile([128, 512], f32, tag="A", bufs=8, name="v_ps")
                for k in range(8):
                    nc.tensor.matmul(
                        out=v_ps[:, 0:128],
                        lhsT=xT_sb[k][:, ut * 128:(ut + 1) * 128],
                        rhs=wv_sb[k], start=(k == 0), stop=(k == 7))
                nc.vector.tensor_scalar_mul(bvm, bv_bc, indc[:, ut:ut + 1])
                nc.vector.scalar_tensor_tensor(
                    out=v_view[:, ut, :, 64:128],
                    in0=v_ps[:, 0:128].rearrange("p (g c) -> p g c", g=2),
                    scalar=indc[:, ut:ut + 1],
                    in1=bvm.rearrange("p (g c) -> p g c", g=2),
                    op0=MUL, op1=mybir.AluOpType.add)

            # ---- attention + out projection -------------------------------
            attnT = actp.tile([128, 8 * T], bf16, tag="attnT")
            attnT_v = attnT.rearrange("p (k t) -> p k t", k=8)
            qvs = [qT_sb[g].rearrange("p (i t) -> p i t", i=4) for g in range(2)]

            def scores_j(mt, gg, j):
                """One score j-chunk: 2 row-packed MMs + exp + band mask,
                one single-bank PSUM tile and one p2 tile per kv-half."""
                qcol = mt * 128
                p2s = []
                for half in range(2):
                    s1 = ps.tile([128, 512], f32, tag="A", bufs=8, name="s1")
                    nc.tensor.matmul(
                        out=s1,
                        lhsT=kT_sb[half * 64:(half + 1) * 64,
                                   qcol + j * 128:qcol + (j + 1) * 128],
                        rhs=qvs[gg][half * 64:(half + 1) * 64, :,
                                    qcol:qcol + 128],
                        start=True, stop=True,
                        tile_position=(64 * half, 0))
                    p2 = attnp.tile([128, 512], bf16, tag="p2", bufs=12,
                                    name="p2")
                    nc.scalar.activation(out=p2, in_=s1, func=Exp)
                    if j == 0:
                        nc.vector.tensor_mul(p2, p2, mA8)
                    elif j == 2:
                        nc.vector.tensor_mul(p2, p2, mB8)
                    p2s.append(p2)
                return p2s

            def pv(mt, gg, p2s):
                """Flipped probs@[V|ones]: output in attnT orientation with
                replicated denominators; normalize + scatter into attnT."""
                qcol = mt * 128
                for half in range(2):
                    o_ps = ps.tile([128, 512], f32, tag="A", bufs=8,
                                   name="o_ps")
                    for j in range(3):
                        nc.tensor.matmul(
                            out=o_ps,
                            lhsT=v_view[:, mt + j, half, :],
                            rhs=p2s[j][half],
                            start=(j == 0), stop=(j == 2))
                    rc = attnp.tile([64, 512], f32, tag="rc", bufs=4,
                                    name="rc")
                    nc.vector.reciprocal_approx_fast(out=rc,
                                                     in_=o_ps[0:64, :])
                    k0 = 2 * gg + 4 * half
                    num = o_ps[64:128, :].rearrange("p (c e t) -> p c e t",
                                                    c=2, e=2)
                    rcv = rc.rearrange("p (c e t) -> p c e t", c=2, e=2)
                    for e in range(2):
                        nc.vector.scalar_tensor_tensor(
                            out=attnT_v[64 * e:64 * e + 64, k0:k0 + 2,
                                        qcol:qcol + 128],
                            in0=num[:, :, e, :], scalar=1.0,
                            in1=rcv[:, :, e, :], op0=MUL, op1=MUL)

            def oproj(mt):
                qcol = mt * 128
                o2s = [ps.tile([128, 512], f32, tag="A", bufs=8, name="o2")
                       for _ in range(2)]
                for k in range(8):
                    for n in range(2):
                        nc.tensor.matmul(
                            out=o2s[n],
                            lhsT=attnT[:, k * T + qcol:k * T + qcol + 128],
                            rhs=wo_sb[k][:, n * 512:(n + 1) * 512],
                            start=(k == 0), stop=(k == 7))
                out_t = attnp.tile([128, DIM], bf16, tag="outt", bufs=2,
                                   name="out_t")
                nc.scalar.copy(out=out_t[:, 0:512], in_=o2s[0])
                nc.vector.tensor_copy(out=out_t[:, 512:1024], in_=o2s[1])
                nc.sync.dma_start(out=out[qcol:qcol + 128, :], in_=out_t)

            # software-pipelined at j-chunk granularity: the PSUM score ring
            # (2 tiles) forces each score MM to wait for the exp two steps
            # back; the previous qtile's PV-gg1 and out-projection are
            # emitted between score steps so the in-order PE queue always
            # has dense matmul work while exps drain.
            prev_p2g1 = None
            prev = None
            for mt in range(8):
                g0 = [scores_j(mt, 0, 0), scores_j(mt, 0, 1)]
                if prev is not None:
                    pv(prev, 1, prev_p2g1)
                g0.append(scores_j(mt, 0, 2))
                if prev is not None:
                    oproj(prev)
                g1 = [scores_j(mt, 1, 0), scores_j(mt, 1, 1)]
                g1.append(scores_j(mt, 1, 2))
                pv(mt, 0, g0)
                prev_p2g1 = g1
                prev = mt
            pv(prev, 1, prev_p2g1)
            oproj(prev)

    nc.compile()
    return nc


def _host_prep(x, Wq, bq, Wk, bk, Wv, bv, Wo, bo):
    import ml_dtypes
    bf16 = ml_dtypes.bfloat16

    # permute Wq columns so qT m-tile holds head m on partitions 0-63 and
    # head m+8 on partitions 64-127 (row-packed score matmuls)
    idx = np.empty(DIM, dtype=np.int64)
    for m in range(8):
        for j in range(128):
            h = m if j < 64 else m + 8
            idx[m * 128 + j] = h * D + (j % 64)
    wq_p = np.ascontiguousarray(Wq[:, idx]).astype(bf16)
    bq_p = bq[idx].astype(np.float32).reshape(8, 128).T.copy()  # (128, 8)

    # permute Wo rows to match the flipped-PV attnT layout:
    # attnT row r = k*128 + p with k = 2gg + 4half + c, e = p//64, d = p%64,
    # head h = 4gg + 8half + 2c + e, original row h*64 + d.
    oidx = np.empty(DIM, dtype=np.int64)
    for k in range(8):
        half, gg, c = k // 4, (k % 4) // 2, k % 2
        for p in range(128):
            e, d = p // 64, p % 64
            h = 4 * gg + 8 * half + 2 * c + e
            oidx[k * 128 + p] = h * D + d
    wo_p = np.ascontiguousarray(Wo[oidx, :]).astype(bf16)

    wk_b = np.ascontiguousarray(Wk).astype(bf16)
    wv_b = np.ascontiguousarray(Wv).astype(bf16)
    bk_c = bk.reshape(KV * D, 1).astype(np.float32)
    bv_r = bv.reshape(1, KV * D).astype(np.float32)

    in_maps = []
    for c in range(NCORES):
        b, qt = c // QT, c % QT
        lo, hi = qt * T - HW, qt * T + T + HW
        xs = np.zeros((TH, DIM), dtype=np.float32)
        s0, s1 = max(lo, 0), min(hi, S)
        xs[s0 - lo:s1 - lo] = x[b, s0:s1]
        ind_r = np.zeros((1, TH), dtype=bf16)
        ind_r[0, s0 - lo:s1 - lo] = 1.0
        ind_c = np.asarray(ind_r, dtype=np.float32).reshape(NU, 128).T.copy()
        in_maps.append({
            "xT": np.ascontiguousarray(xs.T).astype(bf16),
            "Wq": wq_p, "Wk": wk_b, "Wv": wv_b, "Wo": wo_p,
            "bqc": bq_p, "bk_col": bk_c, "bv_row": bv_r,
            "ind": ind_r, "ind_col": ind_c,
        })
    return in_maps


def kernel(x, Wq, bq, Wk, bk, Wv, bv, Wo, bo):
    from concourse.bass_utils import run_bass_kernel_spmd

    x, Wq, bq, Wk, bk, Wv, bv, Wo, bo = (
        np.asarray(a, dtype=np.float32)
        for a in (x, Wq, bq, Wk, bk, Wv, bv, Wo, bo))
    nc = _build_nc()
    in_maps = _host_prep(x, Wq, bq, Wk, bk, Wv, bv, Wo, bo)
    res = run_bass_kernel_spmd(nc, in_maps, core_ids=list(range(NCORES)))
    out = np.empty((B, S, DIM), dtype=np.float32)
    for c in range(NCORES):
        b, qt = c // QT, c % QT
        out[b, qt * T:(qt + 1) * T] = res.results[c]["out"].astype(np.float32)
    out += bo  # output bias is purely additive after the last matmul
    return out


# revision 21
# speedup vs baseline: 1.0215x; 1.0215x over previous
"""Trainium2 Bass kernel for sliding-window GQA attention block (v2).

Reference computation (B=2, S=4096, DIM=1024, H=16 q-heads, KV=2 kv-heads,
D=64, W=256 window):
    q = x@Wq + bq ; k = x@Wk + bk ; v = x@Wv + bv        (GQA repeat kv x8)
    local attention: query t attends keys [t-128, t+128) (zero-padded edges,
    no 1/sqrt(d) scaling), softmax, out = probs@v
    y = out@Wo + bo

Sharding: 8 cores = batch(2) x seq-quarter(4). Each core computes 1024
query rows end-to-end (all 16 heads) from a 1280-row haloed x slice.
No cross-core communication; host pads/transposes/gathers; bo is added
on the host (purely additive after the last matmul).

v2 structure (vs baseline):
  - K/V/Q projections run k-chunk-outer so the PE starts as soon as the
    first xT/wq DMA chunk lands; K is computed directly transposed with
    wk stationary; biases fold into DVE scalar_tensor_tensor ops (no K=1
    bias matmuls).
  - scores computed transposed as before (keys on partitions, kv-halves
    row-packed via tile_position so the two K=64 matmuls overlap).
  - probs@V is FLIPPED: the stationary is [V(64) | ones(64)] so the
    output lands directly in attnT orientation (head-dim on partitions)
    with the softmax denominator replicated across partitions 64-127 --
    no PE transposes, no partition broadcasts. Normalization is a DVE
    reciprocal_approx_fast on the replicated denominator + fused
    multiply into attnT.
  - the middle score chunk (always fully in-window) skips the band-mask
    multiply entirely.
  - out projection accumulates in PSUM and DMAs PSUM->DRAM directly.
"""

import functools
import numpy as np

B, S, DIM = 2, 4096, 1024
H, KV, D = 16, 2, 64
W, HW = 256, 128
NCORES = 8
QT = 4           # sequence quarters
T = S // QT      # 1024 query rows per core
TH = T + 2 * HW  # 1280 haloed rows
NU = TH // 128   # 10 key/value u-tiles


@functools.lru_cache(maxsize=1)
def _build_nc():
    import concourse.bacc as bacc
    import concourse.tile as tile
    from concourse import mybir

    f32 = mybir.dt.float32
    bf16 = mybir.dt.bfloat16
    Exp = mybir.ActivationFunctionType.Exp
    Identity = mybir.ActivationFunctionType.Identity
    MUL = mybir.AluOpType.mult

    nc = bacc.Bacc("TRN2", target_bir_lowering=False, debug=False)

    xT = nc.dram_tensor("xT", [DIM, TH], bf16, kind="ExternalInput")
    wq = nc.dram_tensor("Wq", [DIM, DIM], bf16, kind="ExternalInput")
    wk = nc.dram_tensor("Wk", [DIM, KV * D], bf16, kind="ExternalInput")
    wv = nc.dram_tensor("Wv", [DIM, KV * D], bf16, kind="ExternalInput")
    wo = nc.dram_tensor("Wo", [DIM, DIM], bf16, kind="ExternalInput")
    bqc = nc.dram_tensor("bqc", [128, 8], f32, kind="ExternalInput")
    bk_col = nc.dram_tensor("bk_col", [128, 1], f32, kind="ExternalInput")
    bv_row = nc.dram_tensor("bv_row", [1, KV * D], f32, kind="ExternalInput")
    ind = nc.dram_tensor("ind", [1, TH], bf16, kind="ExternalInput")
    ind_col = nc.dram_tensor("ind_col", [128, NU], f32, kind="ExternalInput")
    out = nc.dram_tensor("out", [T, DIM], bf16, kind="ExternalOutput")

    with tile.TileContext(nc) as tc:
        with tc.tile_pool(name="const", bufs=1) as const, \
             tc.tile_pool(name="w", bufs=1) as wpool, \
             tc.tile_pool(name="act", bufs=1) as actp, \
             tc.tile_pool(name="attn", bufs=2) as attnp, \
             tc.tile_pool(name="ps", bufs=2, space="PSUM") as ps:

            # ---- weight/activation loads ----------------------------------
            # descriptor generation (~650ns per dma_start) serializes per
            # sequencer queue, so the critical first chunks (xT[k], wq[k],
            # then wk) round-robin across three DMA-capable queues; wv/wo
            # and small consts follow.
            qs = [nc.sync, nc.scalar, nc.gpsimd]
            xT_sb, wq_sb, wk_sb, wv_sb, wo_sb = [], [], [], [], []
            for k in range(8):
                t_x = wpool.tile([128, TH], bf16, tag=f"xT{k}", name=f"xT{k}")
                qs[(2 * k) % 3].dma_start(out=t_x,
                                          in_=xT[k * 128:(k + 1) * 128, :])
                xT_sb.append(t_x)
                t_q = wpool.tile([128, DIM], bf16, tag=f"wq{k}", name=f"wq{k}")
                qs[(2 * k + 1) % 3].dma_start(out=t_q,
                                              in_=wq[k * 128:(k + 1) * 128, :])
                wq_sb.append(t_q)
            for k in range(8):
                t_k = wpool.tile([128, KV * D], bf16, tag=f"wk{k}", name=f"wk{k}")
                qs[k % 3].dma_start(out=t_k, in_=wk[k * 128:(k + 1) * 128, :])
                wk_sb.append(t_k)
            bq_sb = const.tile([128, 8], f32, tag="bq")
            nc.scalar.dma_start(out=bq_sb, in_=bqc[:, :])
            bkc = const.tile([128, 1], f32, tag="bkc")
            nc.sync.dma_start(out=bkc, in_=bk_col[:, :])
            ind_sb = const.tile([1, TH], bf16, tag="ind")
            nc.gpsimd.dma_start(out=ind_sb, in_=ind[:, :])
            for k in range(8):
                t_v = wpool.tile([128, KV * D], bf16, tag=f"wv{k}", name=f"wv{k}")
                qs[k % 3].dma_start(out=t_v, in_=wv[k * 128:(k + 1) * 128, :])
                wv_sb.append(t_v)
                t_o = wpool.tile([128, DIM], bf16, tag=f"wo{k}", name=f"wo{k}")
                qs[(k + 1) % 3].dma_start(out=t_o,
                                          in_=wo[k * 128:(k + 1) * 128, :])
                wo_sb.append(t_o)
            bvr = const.tile([1, KV * D], f32, tag="bvr")
            nc.scalar.dma_start(out=bvr, in_=bv_row[:, :])
            indc = const.tile([128, NU], f32, tag="indc")
            nc.sync.dma_start(out=indc, in_=ind_col[:, :])
            # broadcast rows for column-varying bias/mask folds
            ind_bc = const.tile([128, TH], bf16, tag="ind_bc")
            nc.gpsimd.partition_broadcast(out_ap=ind_bc, in_ap=ind_sb)
            bv_bc = const.tile([128, KV * D], f32, tag="bv_bc")
            nc.gpsimd.partition_broadcast(out_ap=bv_bc, in_ap=bvr)

            # 0/1 band masks, transposed orientation (key partition r, query
            # col c), full 1024 wide = 8 blocks of 128 (4 head-blocks per
            # kv-half). Chunk j=0 valid where r >= c; j=2 valid where r < c;
            # j=1 is always fully valid and is never masked. Built on the
            # (head-phase-idle) DVE so the gpsimd queue stays free for DMA.
            mA8 = const.tile([128, 512], bf16, tag="mA8")
            mB8 = const.tile([128, 512], bf16, tag="mB8")
            nc.gpsimd.memset(mA8, 1.0)
            nc.gpsimd.memset(mB8, 1.0)
            for blk in range(4):
                nc.gpsimd.affine_select(
                    out=mA8[:, blk * 128:(blk + 1) * 128],
                    in_=mA8[:, blk * 128:(blk + 1) * 128],
                    compare_op=mybir.AluOpType.is_ge,
                    fill=0.0, base=0, pattern=[[-1, 128]],
                    channel_multiplier=1)
                nc.gpsimd.affine_select(
                    out=mB8[:, blk * 128:(blk + 1) * 128],
                    in_=mB8[:, blk * 128:(blk + 1) * 128],
                    compare_op=mybir.AluOpType.is_ge,
                    fill=0.0, base=-1, pattern=[[1, 128]],
                    channel_multiplier=-1)

            # ---- PE warmup: the HAM clock-gate starts at 1.2 GHz and the
            # first input chunks take ~7us to land. Dummy matmuls on memset
            # tiles fill the DMA wait and un-throttle the PE to 2.4 GHz
            # before real work arrives.
            wu_l = const.tile([128, 128], bf16, tag="wu_l")
            wu_r = const.tile([128, 512], bf16, tag="wu_r")
            nc.vector.memset(wu_l, 0.5)
            nc.vector.memset(wu_r, 0.5)
            wu_ps = ps.tile([128, 512], f32, tag="A", bufs=8, name="wu_ps")
            for _ in range(24):
                nc.tensor.matmul(out=wu_ps, lhsT=wu_l, rhs=wu_r,
                                 start=True, stop=True)

            # ---- Q projection: qT tile g holds heads (m, m+8) on partition
            # halves for m = 4g..4g+3 (column-permuted Wq does the packing).
            # k-chunk-outer in groups of 4 m so the PE consumes xT/wq DMA
            # chunks as they arrive.
            qT_sb = [actp.tile([128, 4 * T], bf16, tag=f"qT{g}", name=f"qT{g}")
                     for g in range(2)]

            def q_group(grp):                    # m in [4*grp, 4*grp+4)
                pa = [ps.tile([128, 512], f32, tag="A", bufs=8,
                              name=f"qA{grp}{i}") for i in range(8)]
                for k in range(8):
                    for mi in range(4):
                        m = 4 * grp + mi
                        for n in range(2):
                            nc.tensor.matmul(
                                out=pa[2 * mi + n],
                                lhsT=wq_sb[k][:, m * 128:(m + 1) * 128],
                                rhs=xT_sb[k][:, HW + n * 512: HW + (n + 1) * 512],
                                start=(k == 0), stop=(k == 7))
                for mi in range(4):
                    m = 4 * grp + mi
                    off = (m % 4) * T
                    for n in range(2):
                        nc.scalar.activation(
                            out=qT_sb[grp][:, off + n * 512:off + (n + 1) * 512],
                            in_=pa[2 * mi + n],
                            func=Identity, bias=bq_sb[:, m:m + 1], scale=1.0)

            # ---- K projection, directly transposed (kv*64+d on partitions,
            # token on free). wk stationary, xT moving; bias-add and halo
            # zeroing fused into the DVE copy. Emitted between the two Q
            # groups so the PE has work while Q-grp0's ACT copies drain.
            kT_sb = actp.tile([128, TH], bf16, tag="kT")

            def k_proj():
                k_ps = [ps.tile([128, 512], f32, tag="A", bufs=8,
                                name=f"kp{c}") for c in range(3)]
                k_dst = [k_ps[0][:, :], k_ps[1][:, :], k_ps[2][:, 0:256]]
                k_w = [512, 512, 256]
                for k in range(8):
                    for c in range(3):
                        nc.tensor.matmul(
                            out=k_dst[c], lhsT=wk_sb[k],
                            rhs=xT_sb[k][:, c * 512:c * 512 + k_w[c]],
                            start=(k == 0), stop=(k == 7))
                for c in range(3):
                    nc.vector.scalar_tensor_tensor(
                        out=kT_sb[:, c * 512:c * 512 + k_w[c]],
                        in0=k_dst[c], scalar=bkc[:, 0:1],
                        in1=ind_bc[:, c * 512:c * 512 + k_w[c]],
                        op0=mybir.AluOpType.add, op1=MUL)

            q_group(0)
            k_proj()
            q_group(1)

            # ---- V projection (keys on partitions). v_sb u-tile layout per
            # kv-half g: [ones (64) | V (64)]; the 64 ones columns make the
            # flipped probs@[1|V] matmul emit the softmax denominator
            # REPLICATED on output partitions 0-63 (base 0, required by
            # reciprocal_approx_fast). ut-outer / k-inner with one PSUM tile
            # per ut: interleaved accumulation groups must not share a PSUM
            # bank (start=True clears the whole bank).
            v_sb = actp.tile([128, NU * 256], bf16, tag="V")
            v_view = v_sb.rearrange("p (u g c) -> p u g c", u=NU, g=2)
            nc.vector.memset(v_view[:, :, :, 0:64], 1.0)
            bvm = attnp.tile([128, KV * D], f32, tag="bvm", bufs=2)
            for ut in range(NU):
                v_ps = ps.tile([128, 512], f32, tag="A", bufs=8, name="v_ps")
                for k in range(8):
                    nc.tensor.matmul(
                        out=v_ps[:, 0:128],
                        lhsT=xT_sb[k][:, ut * 128:(ut + 1) * 128],
                        rhs=wv_sb[k], start=(k == 0), stop=(k == 7))
                nc.vector.tensor_scalar_mul(bvm, bv_bc, indc[:, ut:ut + 1])
                nc.vector.scalar_tensor_tensor(
                    out=v_view[:, ut, :, 64:128],
                    in0=v_ps[:, 0:128].rearrange("p (g c) -> p g c", g=2),
                    scalar=indc[:, ut:ut + 1],
                    in1=bvm.rearrange("p (g c) -> p g c", g=2),
                    op0=MUL, op1=mybir.AluOpType.add)

            # ---- attention + out projection -------------------------------
            attnT = actp.tile([128, 8 * T], bf16, tag="attnT")
            attnT_v = attnT.rearrange("p (k t) -> p k t", k=8)
            qvs = [qT_sb[g].rearrange("p (i t) -> p i t", i=4) for g in range(2)]

            def scores_j(mt, gg, j):
                """One score j-chunk: 2 row-packed MMs + exp + band mask,
                one single-bank PSUM tile and one p2 tile per kv-half."""
                qcol = mt * 128
                p2s = []
                for half in range(2):
                    s1 = ps.tile([128, 512], f32, tag="A", bufs=8, name="s1")
                    nc.tensor.matmul(
                        out=s1,
                        lhsT=kT_sb[half * 64:(half + 1) * 64,
                                   qcol + j * 128:qcol + (j + 1) * 128],
                        rhs=qvs[gg][half * 64:(half + 1) * 64, :,
                                    qcol:qcol + 128],
                        start=True, stop=True,
                        tile_position=(64 * half, 0))
                    p2 = attnp.tile([128, 512], bf16, tag="p2", bufs=12,
                                    name="p2")
                    nc.scalar.activation(out=p2, in_=s1, func=Exp)
                    if j == 0:
                        nc.vector.tensor_mul(p2, p2, mA8)
                    elif j == 2:
                        nc.vector.tensor_mul(p2, p2, mB8)
                    p2s.append(p2)
                return p2s

            def pv(mt, gg, p2s):
                """Flipped probs@[V|ones]: output in attnT orientation with
                replicated denominators; normalize + scatter into attnT."""
                qcol = mt * 128
                for half in range(2):
                    o_ps = ps.tile([128, 512], f32, tag="A", bufs=8,
                                   name="o_ps")
                    for j in range(3):
                        nc.tensor.matmul(
                            out=o_ps,
                            lhsT=v_view[:, mt + j, half, :],
                            rhs=p2s[j][half],
                            start=(j == 0), stop=(j == 2))
                    rc = attnp.tile([64, 512], f32, tag="rc", bufs=4,
                                    name="rc")
                    nc.vector.reciprocal_approx_fast(out=rc,
                                                     in_=o_ps[0:64, :])
                    k0 = 2 * gg + 4 * half
                    num = o_ps[64:128, :].rearrange("p (c e t) -> p c e t",
                                                    c=2, e=2)
                    rcv = rc.rearrange("p (c e t) -> p c e t", c=2, e=2)
                    for e in range(2):
                        nc.vector.scalar_tensor_tensor(
                            out=attnT_v[64 * e:64 * e + 64, k0:k0 + 2,
                                        qcol:qcol + 128],
                            in0=num[:, :, e, :], scalar=1.0,
                            in1=rcv[:, :, e, :], op0=MUL, op1=MUL)

            def oproj(mt):
                qcol = mt * 128
                o2s = [ps.tile([128, 512], f32, tag="A", bufs=8, name="o2")
                       for _ in range(2)]
                for k in range(8):
                    for n in range(2):
                        nc.tensor.matmul(
                            out=o2s[n],
                            lhsT=attnT[:, k * T + qcol:k * T + qcol + 128],
                            rhs=wo_sb[k][:, n * 512:(n + 1) * 512],
                            start=(k == 0), stop=(k == 7))
                out_t = attnp.tile([128, DIM], bf16, tag="outt", bufs=2,
                                   name="out_t")
                nc.scalar.copy(out=out_t[:, 0:512], in_=o2s[0])
                nc.vector.tensor_copy(out=out_t[:, 512:1024], in_=o2s[1])
                nc.sync.dma_start(out=out[qcol:qcol + 128, :], in_=out_t)

            # software-pipelined at j-chunk granularity: the PSUM score ring
            # (2 tiles) forces each score MM to wait for the exp two steps
            # back; the previous qtile's PV-gg1 and out-projection are
            # emitted between score steps so the in-order PE queue always
            # has dense matmul work while exps drain.
            prev_p2g1 = None
            prev = None
            for mt in range(8):
                g0 = [scores_j(mt, 0, 0), scores_j(mt, 0, 1)]
                if prev is not None:
                    pv(prev, 1, prev_p2g1)
                g0.append(scores_j(mt, 0, 2))
                if prev is not None:
                    oproj(prev)
                g1 = [scores_j(mt, 1, 0), scores_j(mt, 1, 1)]
                g1.append(scores_j(mt, 1, 2))
                pv(mt, 0, g0)
                prev_p2g1 = g1
                prev = mt
            pv(prev, 1, prev_p2g1)
            oproj(prev)

    nc.compile()
    return nc


def _host_prep(x, Wq, bq, Wk, bk, Wv, bv, Wo, bo):
    import ml_dtypes
    bf16 = ml_dtypes.bfloat16

    # permute Wq columns so qT m-tile holds head m on partitions 0-63 and
    # head m+8 on partitions 64-127 (row-packed score matmuls)
    idx = np.empty(DIM, dtype=np.int64)
    for m in range(8):
        for j in range(128):
            h = m if j < 64 else m + 8
            idx[m * 128 + j] = h * D + (j % 64)
    wq_p = np.ascontiguousarray(Wq[:, idx]).astype(bf16)
    bq_p = bq[idx].astype(np.float32).reshape(8, 128).T.copy()  # (128, 8)

    # permute Wo rows to match the flipped-PV attnT layout:
    # attnT row r = k*128 + p with k = 2gg + 4half + c, e = p//64, d = p%64,
    # head h = 4gg + 8half + 2c + e, original row h*64 + d.
    oidx = np.empty(DIM, dtype=np.int64)
    for k in range(8):
        half, gg, c = k // 4, (k % 4) // 2, k % 2
        for p in range(128):
            e, d = p // 64, p % 64
            h = 4 * gg + 8 * half + 2 * c + e
            oidx[k * 128 + p] = h * D + d
    wo_p = np.ascontiguousarray(Wo[oidx, :]).astype(bf16)

    wk_b = np.ascontiguousarray(Wk).astype(bf16)
    wv_b = np.ascontiguousarray(Wv).astype(bf16)
    bk_c = bk.reshape(KV * D, 1).astype(np.float32)
    bv_r = bv.reshape(1, KV * D).astype(np.float32)

    in_maps = []
    for c in range(NCORES):
        b, qt = c // QT, c % QT
        lo, hi = qt * T - HW, qt * T + T + HW
        xs = np.zeros((TH, DIM), dtype=np.float32)
        s0, s1 = max(lo, 0), min(hi, S)
        xs[s0 - lo:s1 - lo] = x[b, s0:s1]
        ind_r = np.zeros((1, TH), dtype=bf16)
        ind_r[0, s0 - lo:s1 - lo] = 1.0
        ind_c = np.asarray(ind_r, dtype=np.float32).reshape(NU, 128).T.copy()
        in_maps.append({
            "xT": np.ascontiguousarray(xs.T).astype(bf16),
            "Wq": wq_p, "Wk": wk_b, "Wv": wv_b, "Wo": wo_p,
            "bqc": bq_p, "bk_col": bk_c, "bv_row": bv_r,
            "ind": ind_r, "ind_col": ind_c,
        })
    return in_maps


def kernel(x, Wq, bq, Wk, bk, Wv, bv, Wo, bo):
    from concourse.bass_utils import run_bass_kernel_spmd

    x, Wq, bq, Wk, bk, Wv, bv, Wo, bo = (
        np.asarray(a, dtype=np.float32)
        for a in (x, Wq, bq, Wk, bk, Wv, bv, Wo, bo))
    nc = _build_nc()
    in_maps = _host_prep(x, Wq, bq, Wk, bk, Wv, bv, Wo, bo)
    res = run_bass_kernel_spmd(nc, in_maps, core_ids=list(range(NCORES)))
    out = np.empty((B, S, DIM), dtype=np.float32)
    for c in range(NCORES):
        b, qt = c // QT, c % QT
        out[b, qt * T:(qt + 1) * T] = res.results[c]["out"].astype(np.float32)
    out += bo  # output bias is purely additive after the last matmul
    return out
